# revision 3
# baseline (speedup 1.0000x reference)
"""Trainium2 Bass kernel for nn_AttentionMechanism (tanh-MLP attention).

Math (per batch b):
  q[:, b]   = W_h_w @ h_t[b] + W_h_b + W_b                  (host, tiny)
  U[beta,s,b] = sum_c W_w[beta,c] V[c,s,b] + q[beta,b]       (PE)
  T = tanh(U)                                                (ACT)
  E[s,b]    = sum_beta bw[beta] T[beta,s,b]                  (PE, output replicated over partitions)
  w = exp(E)        (no max-subtraction needed: |E| <= ||bw||_1 ~ 8)
  P[c,b]    = sum_s w[s,b] V[c,s,b]                          (DVE mul+reduce)
  SE[b]     = sum_s w[s,b]                                   (DVE reduce)
  C[b,0,c]  = sum_cores P / sum_cores SE                     (host, tiny)

Sharding: position-parallel across 8 cores (hp dim, 8 rows each) -> each
core gets a contiguous 32MB slice of V; softmax combined on host.

On-chip layout: V tiles [c=128 part, (s=128, b=64) free] - DMA reads
256B-contiguous chunks.  Matmul rhs enumerates (b-outer, s-inner) so all
downstream free layouts are (b, s).  The per-batch bias q is folded into
the U matmul via an extra K=64 pass with a one-hot "indicator" rhs
(identity matrix broadcast over s with a step-0 AP).
"""

import sys
from contextlib import ExitStack

import numpy as np

if "/opt/trn_rl_repo" not in sys.path:
    sys.path.insert(0, "/opt/trn_rl_repo")

import ml_dtypes

HP, WP, C_DIM, B = 64, 64, 256, 64
BETA, HIDDEN = 512, 512
NCORES = 8
S_CORE = (HP // NCORES) * WP  # 512 positions per core
S_TILE = 128                  # positions per SBUF tile
B_SUP = 16                    # batches per supergroup (PSUM span 2048)
B_MM = 4                      # batches per matmul (N = 4*128 = 512)

_NC_CACHE = {}


def _build_nc(s_core=S_CORE):
    import concourse.bass as bass
    import concourse.bacc as bacc
    import concourse.tile as tile
    import concourse.mybir as mybir
    from concourse.mybir import dt

    AF = mybir.ActivationFunctionType
    ALU = mybir.AluOpType
    AX = mybir.AxisListType
    f32, bf16, f32r = dt.float32, dt.bfloat16, dt.float32r

    n_st = s_core // S_TILE
    n_sup = B // B_SUP

    nc = bacc.Bacc("TRN2", target_bir_lowering=False, debug=False,
                   num_devices=NCORES)

    v_d = nc.dram_tensor("v", [s_core, C_DIM, B], f32r, kind="ExternalInput")
    wt_d = nc.dram_tensor("wt", [128, 2 * BETA], f32r, kind="ExternalInput")
    qt_d = nc.dram_tensor("qt", [B, BETA], f32r, kind="ExternalInput")
    bwr_d = nc.dram_tensor("bwr", [128, BETA], bf16, kind="ExternalInput")
    ind_d = nc.dram_tensor("ind", [B, B], f32r, kind="ExternalInput")
    p_d = nc.dram_tensor("p_out", [2, 128, B], f32, kind="ExternalOutput")
    se_d = nc.dram_tensor("se_out", [1, B], f32, kind="ExternalOutput")

    with tile.TileContext(nc) as tc, ExitStack() as ctx:
        cpool = ctx.enter_context(tc.tile_pool(name="const", bufs=1))
        vpool = ctx.enter_context(tc.tile_pool(name="vp", bufs=2))
        tpool = ctx.enter_context(tc.tile_pool(name="tp", bufs=5))
        wpool = ctx.enter_context(tc.tile_pool(name="wp", bufs=2))
        ppool = ctx.enter_context(tc.tile_pool(name="pp", bufs=2))
        apool = ctx.enter_context(tc.tile_pool(name="ap", bufs=1))
        fpool = ctx.enter_context(tc.tile_pool(name="fp", bufs=3))
        psum = ctx.enter_context(tc.tile_pool(name="ps", bufs=2, space="PSUM"))

        # ---- constants ----
        wt_sb = cpool.tile([128, 2 * BETA], f32r, tag="wt")
        nc.sync.dma_start(wt_sb, wt_d[:])
        qt_sb = cpool.tile([B, BETA], f32r, tag="qt")
        nc.sync.dma_start(qt_sb, qt_d[:])
        bwr_sb = cpool.tile([128, BETA], bf16, tag="bwr")
        nc.sync.dma_start(bwr_sb, bwr_d[:])
        ind_sb = cpool.tile([B, B], f32r, tag="ind")
        nc.sync.dma_start(ind_sb, ind_d[:])

        # ---- accumulators ----
        p_parts = [apool.tile([128, n_st * B], f32, tag=f"ppart{k}", name=f"ppart{k}")
                   for k in range(2)]
        se_parts = apool.tile([128, n_st * B], f32, tag="separt")

        for st in range(n_st):
            vt = []
            for k in range(2):
                v_sb = vpool.tile([128, S_TILE * B], f32r, tag=f"v{k}",
                                  name=f"v{k}")
                src = v_d[st * S_TILE:(st + 1) * S_TILE,
                          k * 128:(k + 1) * 128, :].rearrange("s c b -> c s b")
                nc.sync.dma_start(v_sb.rearrange("p (s b) -> p s b", b=B), src)
                vt.append(v_sb)
            # views [p, b, s]
            vv = [t.rearrange("p (s b) -> p b s", b=B) for t in vt]
            vvf = [t.bitcast(f32).rearrange("p (s b) -> p b s", b=B) for t in vt]

            for sup in range(n_sup):
                t_tiles = []
                for m in range(4):
                    u = psum.tile([128, 4 * 512], f32, tag="acc", name="u")
                    for t in range(4):
                        b0 = sup * B_SUP + t * B_MM
                        out_sl = u[:, t * 512:(t + 1) * 512]
                        nc.tensor.matmul(
                            out_sl, wt_sb[:, m * 128:(m + 1) * 128],
                            vv[0][:, b0:b0 + B_MM, :], start=True, stop=False)
                        nc.tensor.matmul(
                            out_sl,
                            wt_sb[:, BETA + m * 128:BETA + (m + 1) * 128],
                            vv[1][:, b0:b0 + B_MM, :], start=False, stop=False)
                        rhs_q = ind_sb[:, b0:b0 + B_MM] \
                            .broadcast_to([B, B_MM, S_TILE])
                        nc.tensor.matmul(
                            out_sl, qt_sb[:, m * 128:(m + 1) * 128],
                            rhs_q, start=False, stop=True)
                    t_m = tpool.tile([128, 4 * 512], bf16, tag="t", name="t_m")
                    nc.scalar.activation(t_m, u, AF.Tanh)
                    t_tiles.append(t_m)

                e_rep = psum.tile([128, 4 * 512], f32, tag="acc", name="e_rep")
                for t in range(4):
                    for m in range(4):
                        nc.tensor.matmul(
                            e_rep[:, t * 512:(t + 1) * 512],
                            bwr_sb[:, m * 128:(m + 1) * 128],
                            t_tiles[m][:, t * 512:(t + 1) * 512],
                            start=(m == 0), stop=(m == 3))
                w_rep = wpool.tile([128, 4 * 512], bf16, tag="w", name="w_rep")
                nc.scalar.activation(w_rep, e_rep, AF.Exp)

                wv = w_rep.rearrange("p (b s) -> p b s", s=S_TILE)
                col = st * B + sup * B_SUP
                for k in range(2):
                    prod = ppool.tile([128, B_SUP * S_TILE], f32, tag="prod",
                                      name="prod")
                    pv = prod.rearrange("p (b s) -> p b s", s=S_TILE)
                    nc.vector.tensor_mul(
                        pv, vvf[k][:, sup * B_SUP:(sup + 1) * B_SUP, :], wv)
                    nc.vector.tensor_reduce(
                        p_parts[k][:, col:col + B_SUP], pv, axis=AX.X, op=ALU.add)
                nc.vector.tensor_reduce(
                    se_parts[:, col:col + B_SUP], wv, axis=AX.X, op=ALU.add)

        # ---- final reduction over s-tiles & DMA out ----
        for k in range(2):
            pf = fpool.tile([128, B], f32, tag="fin", name="pf")
            nc.vector.tensor_reduce(
                pf, p_parts[k].rearrange("p (st b) -> p b st", b=B),
                axis=AX.X, op=ALU.add)
            nc.sync.dma_start(p_d[k], pf)
        sf = fpool.tile([128, B], f32, tag="fin", name="sf")
        nc.vector.tensor_reduce(
            sf, se_parts.rearrange("p (st b) -> p b st", b=B),
            axis=AX.X, op=ALU.add)
        nc.sync.dma_start(se_d[:], sf[0:1, :])

    nc.compile()
    return nc


def _get_nc(s_core=S_CORE):
    if s_core not in _NC_CACHE:
        _NC_CACHE[s_core] = _build_nc(s_core)
    return _NC_CACHE[s_core]


def _host_smalls(h_t, W_h_w, W_h_b, W_w, W_b, beta_w):
    q = h_t[:, 0, :].astype(np.float64) @ W_h_w.T.astype(np.float64) \
        + W_h_b + W_b                                  # [b, beta]
    qt = np.ascontiguousarray(q.astype(np.float32))
    wt = np.ascontiguousarray(
        W_w.T.reshape(2, 128, BETA).transpose(1, 0, 2).reshape(128, 2 * BETA)
    ).astype(np.float32)
    bw = beta_w[0].astype(np.float32)
    bwr = np.ascontiguousarray(
        np.repeat(bw.reshape(4, 128).T[:, :, None], 128, axis=2).reshape(128, BETA)
    ).astype(ml_dtypes.bfloat16)
    ind = np.eye(B, dtype=np.float32)
    return qt, wt, bwr, ind


_PROFILE = False
_LAST_PERF = {}


def kernel(**inputs):
    from concourse.bass_utils import run_bass_kernel_spmd

    V = np.asarray(inputs["V"], dtype=np.float32)
    h_t = np.asarray(inputs["h_t"], dtype=np.float32)
    W_h_w = np.asarray(inputs["W_h_w"], dtype=np.float32)
    W_h_b = np.asarray(inputs["W_h_b"], dtype=np.float32)
    W_w = np.asarray(inputs["W_w"], dtype=np.float32)
    W_b = np.asarray(inputs["W_b"], dtype=np.float32)
    beta_w = np.asarray(inputs["beta_w"], dtype=np.float32)
    beta_b = np.asarray(inputs["beta_b"], dtype=np.float32)

    qt, wt, bwr, ind = _host_smalls(h_t, W_h_w, W_h_b, W_w, W_b, beta_w)

    rows = HP // NCORES
    in_maps = []
    for k in range(NCORES):
        vk = np.ascontiguousarray(
            V[k * rows:(k + 1) * rows].reshape(S_CORE, C_DIM, B))
        in_maps.append({"v": vk, "wt": wt, "qt": qt, "bwr": bwr, "ind": ind})

    nc = _get_nc()
    res = run_bass_kernel_spmd(nc, in_maps, core_ids=list(range(NCORES)),
                               trace=_PROFILE)
    if _PROFILE:
        _LAST_PERF["exec_time_ns"] = res.exec_time_ns
        _LAST_PERF["trace"] = res.instructions_and_trace
    P = np.zeros((2, 128, B), np.float64)
    SE = np.zeros((B,), np.float64)
    for r in res.results:
        P += r["p_out"]
        SE += r["se_out"][0]
    P = P.reshape(C_DIM, B)
    # softmax is shift-invariant so beta_b cancels; no max-sub needed (|E|<=~8)
    C = (P / SE).T.reshape(B, 1, C_DIM)
    return C.astype(np.float32)


# revision 5
# speedup vs baseline: 1.0253x; 1.0253x over previous
"""Trainium2 Bass kernel for nn_AttentionMechanism (tanh-MLP attention).

Math (per batch b):
  q[:, b]   = W_h_w @ h_t[b] + W_h_b + W_b                  (host, tiny)
  U[beta,s,b] = sum_c W_w[beta,c] V[c,s,b] + q[beta,b]       (PE)
  T = tanh(U)                                                (ACT)
  E[s,b]    = sum_beta bw[beta] T[beta,s,b]                  (PE, output replicated over partitions)
  w = exp(E)        (no max-subtraction needed: |E| <= ||bw||_1 ~ 8)
  P[c,b]    = sum_s w[s,b] V[c,s,b]                          (DVE mul+reduce)
  SE[b]     = sum_s w[s,b]                                   (DVE reduce)
  C[b,0,c]  = sum_cores P / sum_cores SE                     (host, tiny)

Sharding: position-parallel across 8 cores (hp dim, 8 rows each); softmax
combined on host.  Host pre-lays V out per-core as [c, s, b] bf16 (the
sharding-prep copy), so the device DMA reads 16KB-contiguous runs at full
HBM bandwidth and all matmuls run at bf16 rate.

On-chip: V tiles [c=128 part, (s=128, b=64) free].  Matmul rhs enumerates
(b-outer, s-inner) in groups of 4 batches (N=512).  The per-batch bias q
is folded into the U matmul as an extra K=64 pass with a one-hot
indicator rhs (identity matrix broadcast over s via a step-0 AP).
Weights are held across the 4 PSUM banks of a supergroup (LDWEIGHTS
amortized 4x).
"""

import sys
from contextlib import ExitStack

import numpy as np

if "/opt/trn_rl_repo" not in sys.path:
    sys.path.insert(0, "/opt/trn_rl_repo")

import ml_dtypes

BF16 = ml_dtypes.bfloat16

HP, WP, C_DIM, B = 64, 64, 256, 64
BETA, HIDDEN = 512, 512
NCORES = 8
S_CORE = (HP // NCORES) * WP  # 512 positions per core
S_TILE = 128                  # positions per SBUF tile
B_SUP = 16                    # batches per supergroup (PSUM span 2048)
B_MM = 4                      # batches per matmul (N = 4*128 = 512)

_NC_CACHE = {}


def _build_nc(s_core=S_CORE):
    import concourse.bass as bass
    import concourse.bacc as bacc
    import concourse.tile as tile
    import concourse.mybir as mybir
    from concourse.mybir import dt

    AF = mybir.ActivationFunctionType
    ALU = mybir.AluOpType
    AX = mybir.AxisListType
    f32, bf16 = dt.float32, dt.bfloat16

    n_st = s_core // S_TILE
    n_sup = B // B_SUP

    nc = bacc.Bacc("TRN2", target_bir_lowering=False, debug=False,
                   num_devices=NCORES)

    v_d = nc.dram_tensor("v", [C_DIM, s_core, B], bf16, kind="ExternalInput")
    wt_d = nc.dram_tensor("wt", [128, 2 * BETA], bf16, kind="ExternalInput")
    qt_d = nc.dram_tensor("qt", [B, BETA], bf16, kind="ExternalInput")
    bwr_d = nc.dram_tensor("bwr", [128, BETA], bf16, kind="ExternalInput")
    ind_d = nc.dram_tensor("ind", [B, B], bf16, kind="ExternalInput")
    p_d = nc.dram_tensor("p_out", [2, 128, B], f32, kind="ExternalOutput")
    se_d = nc.dram_tensor("se_out", [1, B], f32, kind="ExternalOutput")

    with tile.TileContext(nc) as tc, ExitStack() as ctx:
        cpool = ctx.enter_context(tc.tile_pool(name="const", bufs=1))
        vpool = ctx.enter_context(tc.tile_pool(name="vp", bufs=2))
        tpool = ctx.enter_context(tc.tile_pool(name="tp", bufs=5))
        wpool = ctx.enter_context(tc.tile_pool(name="wp", bufs=2))
        ppool = ctx.enter_context(tc.tile_pool(name="pp", bufs=2))
        apool = ctx.enter_context(tc.tile_pool(name="ap", bufs=1))
        fpool = ctx.enter_context(tc.tile_pool(name="fp", bufs=3))
        psum = ctx.enter_context(tc.tile_pool(name="ps", bufs=2, space="PSUM"))

        # ---- constants ----
        wt_sb = cpool.tile([128, 2 * BETA], bf16, tag="wt")
        nc.sync.dma_start(wt_sb, wt_d[:])
        qt_sb = cpool.tile([B, BETA], bf16, tag="qt")
        nc.sync.dma_start(qt_sb, qt_d[:])
        bwr_sb = cpool.tile([128, BETA], bf16, tag="bwr")
        nc.sync.dma_start(bwr_sb, bwr_d[:])
        ind_sb = cpool.tile([B, B], bf16, tag="ind")
        nc.sync.dma_start(ind_sb, ind_d[:])

        # ---- accumulators ----
        p_parts = [apool.tile([128, n_st * B], f32, tag=f"ppart{k}", name=f"ppart{k}")
                   for k in range(2)]
        se_parts = apool.tile([128, n_st * B], f32, tag="separt")

        for st in range(n_st):
            vt = []
            for k in range(2):
                v_sb = vpool.tile([128, S_TILE * B], bf16, tag=f"v{k}",
                                  name=f"v{k}")
                src = v_d[k * 128:(k + 1) * 128,
                          st * S_TILE:(st + 1) * S_TILE, :]
                nc.sync.dma_start(v_sb.rearrange("p (s b) -> p s b", b=B), src)
                vt.append(v_sb)
            # views [p, b, s] for matmul rhs, [p, s, b] for the P stage
            vv = [t.rearrange("p (s b) -> p b s", b=B) for t in vt]
            vsb = [t.rearrange("p (s b) -> p s b", b=B) for t in vt]

            for sup in range(n_sup):
                t_tiles = []
                for m in range(4):
                    u = psum.tile([128, 4 * 512], f32, tag="acc", name="u")
                    # weight-reuse order: hold each lhsT across the 4 banks
                    for kp in range(3):
                        for t in range(4):
                            b0 = sup * B_SUP + t * B_MM
                            out_sl = u[:, t * 512:(t + 1) * 512]
                            if kp < 2:
                                nc.tensor.matmul(
                                    out_sl,
                                    wt_sb[:, kp * BETA + m * 128:
                                          kp * BETA + (m + 1) * 128],
                                    vv[kp][:, b0:b0 + B_MM, :],
                                    start=(kp == 0), stop=False)
                            else:
                                rhs_q = ind_sb[:, b0:b0 + B_MM] \
                                    .broadcast_to([B, B_MM, S_TILE])
                                nc.tensor.matmul(
                                    out_sl, qt_sb[:, m * 128:(m + 1) * 128],
                                    rhs_q, start=False, stop=True)
                    t_m = tpool.tile([128, 4 * 512], bf16, tag="t", name="t_m")
                    nc.scalar.activation(t_m, u, AF.Tanh)
                    t_tiles.append(t_m)

                e_rep = psum.tile([128, 4 * 512], f32, tag="acc", name="e_rep")
                for m in range(4):
                    for t in range(4):
                        nc.tensor.matmul(
                            e_rep[:, t * 512:(t + 1) * 512],
                            bwr_sb[:, m * 128:(m + 1) * 128],
                            t_tiles[m][:, t * 512:(t + 1) * 512],
                            start=(m == 0), stop=(m == 3))
                # exp; write w in (s-major, b-inner) layout for the 2x P-mul
                w_rep = wpool.tile([128, 4 * 512], bf16, tag="w", name="w_rep")
                w_out = w_rep.rearrange("p (s t bs) -> p t bs s", t=4, bs=B_MM)
                e_in = e_rep.rearrange("p (t bs s) -> p t bs s", bs=B_MM, s=S_TILE)
                nc.scalar.activation(w_out, e_in, AF.Exp)

                wv = w_rep.rearrange("p (s b) -> p s b", b=B_SUP)
                wvr = w_rep.rearrange("p (s b) -> p b s", b=B_SUP)
                col = st * B + sup * B_SUP
                for k in range(2):
                    prod = ppool.tile([128, B_SUP * S_TILE], bf16, tag="prod",
                                      name="prod")
                    pv = prod.rearrange("p (s b) -> p s b", b=B_SUP)
                    nc.vector.tensor_mul(
                        pv, vsb[k][:, :, sup * B_SUP:(sup + 1) * B_SUP], wv)
                    nc.vector.tensor_reduce(
                        p_parts[k][:, col:col + B_SUP],
                        prod.rearrange("p (s b) -> p b s", b=B_SUP),
                        axis=AX.X, op=ALU.add)
                nc.vector.tensor_reduce(
                    se_parts[:, col:col + B_SUP], wvr, axis=AX.X, op=ALU.add)

        # ---- final reduction over s-tiles & DMA out ----
        for k in range(2):
            pf = fpool.tile([128, B], f32, tag="fin", name="pf")
            nc.vector.tensor_reduce(
                pf, p_parts[k].rearrange("p (st b) -> p b st", b=B),
                axis=AX.X, op=ALU.add)
            nc.sync.dma_start(p_d[k], pf)
        sf = fpool.tile([128, B], f32, tag="fin", name="sf")
        nc.vector.tensor_reduce(
            sf, se_parts.rearrange("p (st b) -> p b st", b=B),
            axis=AX.X, op=ALU.add)
        nc.sync.dma_start(se_d[:], sf[0:1, :])

    nc.compile()
    return nc


def _get_nc(s_core=S_CORE):
    if s_core not in _NC_CACHE:
        _NC_CACHE[s_core] = _build_nc(s_core)
    return _NC_CACHE[s_core]


def _host_smalls(h_t, W_h_w, W_h_b, W_w, W_b, beta_w):
    q = h_t[:, 0, :].astype(np.float64) @ W_h_w.T.astype(np.float64) \
        + W_h_b + W_b                                  # [b, beta]
    qt = np.ascontiguousarray(q).astype(BF16)
    wt = np.ascontiguousarray(
        W_w.T.reshape(2, 128, BETA).transpose(1, 0, 2).reshape(128, 2 * BETA)
    ).astype(BF16)
    bw = beta_w[0].astype(np.float32)
    bwr = np.ascontiguousarray(
        np.repeat(bw.reshape(4, 128).T[:, :, None], 128, axis=2).reshape(128, BETA)
    ).astype(BF16)
    ind = np.eye(B, dtype=np.float32).astype(BF16)
    return qt, wt, bwr, ind


_PROFILE = False
_LAST_PERF = {}


def kernel(**inputs):
    from concourse.bass_utils import run_bass_kernel_spmd

    V = np.asarray(inputs["V"], dtype=np.float32)
    h_t = np.asarray(inputs["h_t"], dtype=np.float32)
    W_h_w = np.asarray(inputs["W_h_w"], dtype=np.float32)
    W_h_b = np.asarray(inputs["W_h_b"], dtype=np.float32)
    W_w = np.asarray(inputs["W_w"], dtype=np.float32)
    W_b = np.asarray(inputs["W_b"], dtype=np.float32)
    beta_w = np.asarray(inputs["beta_w"], dtype=np.float32)
    beta_b = np.asarray(inputs["beta_b"], dtype=np.float32)

    qt, wt, bwr, ind = _host_smalls(h_t, W_h_w, W_h_b, W_w, W_b, beta_w)

    rows = HP // NCORES
    Vb = V.astype(BF16)  # one pass; per-core slices transposed below
    in_maps = []
    for k in range(NCORES):
        # [s, c, b] -> [c, s, b] contiguous (per-core shard layout)
        vk = np.ascontiguousarray(
            Vb[k * rows:(k + 1) * rows].reshape(S_CORE, C_DIM, B)
            .transpose(1, 0, 2))
        in_maps.append({"v": vk, "wt": wt, "qt": qt, "bwr": bwr, "ind": ind})

    nc = _get_nc()
    res = run_bass_kernel_spmd(nc, in_maps, core_ids=list(range(NCORES)),
                               trace=_PROFILE)
    if _PROFILE:
        _LAST_PERF["exec_time_ns"] = res.exec_time_ns
        _LAST_PERF["trace"] = res.instructions_and_trace
    P = np.zeros((2, 128, B), np.float64)
    SE = np.zeros((B,), np.float64)
    for r in res.results:
        P += r["p_out"]
        SE += r["se_out"][0]
    P = P.reshape(C_DIM, B)
    # softmax is shift-invariant so beta_b cancels; no max-sub needed (|E|<=~8)
    C = (P / SE).T.reshape(B, 1, C_DIM)
    return C.astype(np.float32)


# revision 6
# speedup vs baseline: 2.5429x; 2.4801x over previous
"""Trainium2 Bass kernel for nn_AttentionMechanism (tanh-MLP attention).

Math (per batch b):
  q[:, b]   = W_h_w @ h_t[b] + W_h_b + W_b                  (host, tiny)
  U[beta,s,b] = sum_c W_w[beta,c] V[c,s,b] + q[beta,b]       (PE)
  T = tanh(U)                                                (ACT)
  E[s,b]    = sum_beta bw[beta] T[beta,s,b]                  (PE, output replicated over partitions)
  w = exp(E)        (no max-subtraction needed: |E| <= ||bw||_1 ~ 8)
  P[c,b]    = sum_s w[s,b] V[c,s,b]                          (DVE mul+reduce)
  SE[b]     = sum_s w[s,b]                                   (DVE reduce)
  C[b,0,c]  = sum_cores P / sum_cores SE                     (host, tiny)

Sharding: position-parallel across 8 cores (hp dim, 8 rows each); softmax
combined on host.  Host pre-lays V out per-core as [c, b, s] bf16 (the
sharding-prep copy), so the device DMA reads 16KB-contiguous runs at full
HBM bandwidth, every matmul rhs has a contiguous innermost dim (full PE
rate), and the DVE P-stage runs in 2x mode with contiguous reduces.

On-chip: V lives in 8 resident tiles [c-chunk x b-quarter] of
[128, 16*512] bf16; compute is pipelined over b-quarters while later
quarters DMA.  Matmul N=512 tiles are (4 batches x 128 positions).  The
per-batch bias q is folded into the U matmul as an extra K=64 pass with a
one-hot indicator rhs (identity columns broadcast over s via a step-0
AP).  E is computed on the PE with a column-replicated beta_w lhsT so the
exp/P stage gets a partition-replicated w directly.
"""

import sys
from contextlib import ExitStack

import numpy as np

if "/opt/trn_rl_repo" not in sys.path:
    sys.path.insert(0, "/opt/trn_rl_repo")

import ml_dtypes

BF16 = ml_dtypes.bfloat16

HP, WP, C_DIM, B = 64, 64, 256, 64
BETA, HIDDEN = 512, 512
NCORES = 8
S_CORE = (HP // NCORES) * WP  # 512 positions per core
S_CHUNK = 128                 # positions per PSUM n-tile
B_Q = 16                      # batches per quarter (DMA/pipeline unit)
B_MM = 4                      # batches per matmul (N = 4*128 = 512)

_NC_CACHE = {}


def _build_nc(s_core=S_CORE):
    import concourse.bass as bass
    import concourse.bacc as bacc
    import concourse.tile as tile
    import concourse.mybir as mybir
    from concourse.mybir import dt

    AF = mybir.ActivationFunctionType
    ALU = mybir.AluOpType
    AX = mybir.AxisListType
    f32, bf16 = dt.float32, dt.bfloat16

    n_sc = s_core // S_CHUNK
    n_q = B // B_Q

    nc = bacc.Bacc("TRN2", target_bir_lowering=False, debug=False,
                   num_devices=NCORES)

    v_d = nc.dram_tensor("v", [C_DIM, B, s_core], bf16, kind="ExternalInput")
    wt_d = nc.dram_tensor("wt", [128, 2 * BETA], bf16, kind="ExternalInput")
    qt_d = nc.dram_tensor("qt", [B, BETA], bf16, kind="ExternalInput")
    bwr_d = nc.dram_tensor("bwr", [128, BETA], bf16, kind="ExternalInput")
    ind_d = nc.dram_tensor("ind", [B, B], bf16, kind="ExternalInput")
    p_d = nc.dram_tensor("p_out", [2, 128, B], f32, kind="ExternalOutput")
    se_d = nc.dram_tensor("se_out", [1, B], f32, kind="ExternalOutput")

    with tile.TileContext(nc) as tc, ExitStack() as ctx:
        cpool = ctx.enter_context(tc.tile_pool(name="const", bufs=1))
        vpool = ctx.enter_context(tc.tile_pool(name="vp", bufs=1))
        tpool = ctx.enter_context(tc.tile_pool(name="tp", bufs=5))
        wpool = ctx.enter_context(tc.tile_pool(name="wp", bufs=2))
        ppool = ctx.enter_context(tc.tile_pool(name="pp", bufs=2))
        apool = ctx.enter_context(tc.tile_pool(name="ap", bufs=1))
        fpool = ctx.enter_context(tc.tile_pool(name="fp", bufs=3))
        psum = ctx.enter_context(tc.tile_pool(name="ps", bufs=2, space="PSUM"))

        # ---- constants ----
        wt_sb = cpool.tile([128, 2 * BETA], bf16, tag="wt")
        nc.sync.dma_start(wt_sb, wt_d[:])
        qt_sb = cpool.tile([B, BETA], bf16, tag="qt")
        nc.sync.dma_start(qt_sb, qt_d[:])
        bwr_sb = cpool.tile([128, BETA], bf16, tag="bwr")
        nc.sync.dma_start(bwr_sb, bwr_d[:])
        ind_sb = cpool.tile([B, B], bf16, tag="ind")
        nc.sync.dma_start(ind_sb, ind_d[:])

        # ---- V tiles: [c-chunk][b-quarter] resident, DMA'd in q order ----
        vt = [[None, None] for _ in range(n_q)]
        vv = [[None, None] for _ in range(n_q)]
        for q in range(n_q):
            for k in range(2):
                t = vpool.tile([128, B_Q * s_core], bf16, tag=f"v{k}q{q}",
                               name=f"v{k}q{q}")
                nc.sync.dma_start(
                    t, v_d[k * 128:(k + 1) * 128, q * B_Q:(q + 1) * B_Q, :])
                vt[q][k] = t
                vv[q][k] = t.rearrange("p (b s) -> p b s", s=s_core)

        # ---- accumulators ----
        p_parts = [apool.tile([128, n_sc * B], f32, tag=f"ppart{k}",
                              name=f"ppart{k}") for k in range(2)]
        se_parts = apool.tile([128, n_sc * B], f32, tag="separt")

        for q in range(n_q):
            for sc in range(n_sc):
                s0 = sc * S_CHUNK
                t_tiles = []
                for m in range(4):
                    u = psum.tile([128, 4 * 512], f32, tag="acc", name="u")
                    # hold each lhsT across the 4 banks (LDW amortized)
                    for kp in range(3):
                        for t in range(4):
                            out_sl = u[:, t * 512:(t + 1) * 512]
                            if kp < 2:
                                nc.tensor.matmul(
                                    out_sl,
                                    wt_sb[:, kp * BETA + m * 128:
                                          kp * BETA + (m + 1) * 128],
                                    vv[q][kp][:, t * B_MM:(t + 1) * B_MM,
                                              s0:s0 + S_CHUNK],
                                    start=(kp == 0), stop=False)
                            else:
                                b0 = q * B_Q + t * B_MM
                                rhs_q = ind_sb[:, b0:b0 + B_MM] \
                                    .broadcast_to([B, B_MM, S_CHUNK])
                                nc.tensor.matmul(
                                    out_sl, qt_sb[:, m * 128:(m + 1) * 128],
                                    rhs_q, start=False, stop=True)
                    t_m = tpool.tile([128, 4 * 512], bf16, tag="t", name="t_m")
                    nc.scalar.activation(t_m, u, AF.Tanh)
                    t_tiles.append(t_m)

                e_rep = psum.tile([128, 4 * 512], f32, tag="acc", name="e_rep")
                for m in range(4):
                    for t in range(4):
                        nc.tensor.matmul(
                            e_rep[:, t * 512:(t + 1) * 512],
                            bwr_sb[:, m * 128:(m + 1) * 128],
                            t_tiles[m][:, t * 512:(t + 1) * 512],
                            start=(m == 0), stop=(m == 3))
                # w layout (b-quarter-major, s-contig) falls out naturally
                w_rep = wpool.tile([128, 4 * 512], bf16, tag="w", name="w_rep")
                nc.scalar.activation(w_rep, e_rep, AF.Exp)

                wv = w_rep.rearrange("p (b s) -> p b s", b=B_Q)
                col = sc * B + q * B_Q
                for k in range(2):
                    prod = ppool.tile([128, B_Q * S_CHUNK], bf16, tag="prod",
                                      name="prod")
                    pv = prod.rearrange("p (b s) -> p b s", b=B_Q)
                    nc.vector.tensor_mul(
                        pv, vv[q][k][:, :, s0:s0 + S_CHUNK], wv)
                    nc.vector.tensor_reduce(
                        p_parts[k][:, col:col + B_Q], pv,
                        axis=AX.X, op=ALU.add)
                nc.vector.tensor_reduce(
                    se_parts[:, col:col + B_Q], wv, axis=AX.X, op=ALU.add)

        # ---- final reduction over s-chunks & DMA out ----
        for k in range(2):
            pf = fpool.tile([128, B], f32, tag="fin", name="pf")
            nc.vector.tensor_reduce(
                pf, p_parts[k].rearrange("p (sc b) -> p b sc", b=B),
                axis=AX.X, op=ALU.add)
            nc.sync.dma_start(p_d[k], pf)
        sf = fpool.tile([128, B], f32, tag="fin", name="sf")
        nc.vector.tensor_reduce(
            sf, se_parts.rearrange("p (sc b) -> p b sc", b=B),
            axis=AX.X, op=ALU.add)
        nc.sync.dma_start(se_d[:], sf[0:1, :])

    nc.compile()
    return nc


def _get_nc(s_core=S_CORE):
    if s_core not in _NC_CACHE:
        _NC_CACHE[s_core] = _build_nc(s_core)
    return _NC_CACHE[s_core]


def _host_smalls(h_t, W_h_w, W_h_b, W_w, W_b, beta_w):
    q = h_t[:, 0, :].astype(np.float64) @ W_h_w.T.astype(np.float64) \
        + W_h_b + W_b                                  # [b, beta]
    qt = np.ascontiguousarray(q).astype(BF16)
    wt = np.ascontiguousarray(
        W_w.T.reshape(2, 128, BETA).transpose(1, 0, 2).reshape(128, 2 * BETA)
    ).astype(BF16)
    bw = beta_w[0].astype(np.float32)
    bwr = np.ascontiguousarray(
        np.repeat(bw.reshape(4, 128).T[:, :, None], 128, axis=2).reshape(128, BETA)
    ).astype(BF16)
    ind = np.eye(B, dtype=np.float32).astype(BF16)
    return qt, wt, bwr, ind


_PROFILE = False
_LAST_PERF = {}


def kernel(**inputs):
    from concourse.bass_utils import run_bass_kernel_spmd

    V = np.asarray(inputs["V"], dtype=np.float32)
    h_t = np.asarray(inputs["h_t"], dtype=np.float32)
    W_h_w = np.asarray(inputs["W_h_w"], dtype=np.float32)
    W_h_b = np.asarray(inputs["W_h_b"], dtype=np.float32)
    W_w = np.asarray(inputs["W_w"], dtype=np.float32)
    W_b = np.asarray(inputs["W_b"], dtype=np.float32)
    beta_w = np.asarray(inputs["beta_w"], dtype=np.float32)
    beta_b = np.asarray(inputs["beta_b"], dtype=np.float32)

    qt, wt, bwr, ind = _host_smalls(h_t, W_h_w, W_h_b, W_w, W_b, beta_w)

    rows = HP // NCORES
    Vb = V.astype(BF16)
    in_maps = []
    for k in range(NCORES):
        # [s, c, b] -> [c, b, s] contiguous (per-core shard layout)
        vk = np.ascontiguousarray(
            Vb[k * rows:(k + 1) * rows].reshape(S_CORE, C_DIM, B)
            .transpose(1, 2, 0))
        in_maps.append({"v": vk, "wt": wt, "qt": qt, "bwr": bwr, "ind": ind})

    nc = _get_nc()
    res = run_bass_kernel_spmd(nc, in_maps, core_ids=list(range(NCORES)),
                               trace=_PROFILE)
    if _PROFILE:
        _LAST_PERF["exec_time_ns"] = res.exec_time_ns
        _LAST_PERF["trace"] = res.instructions_and_trace
    P = np.zeros((2, 128, B), np.float64)
    SE = np.zeros((B,), np.float64)
    for r in res.results:
        P += r["p_out"]
        SE += r["se_out"][0]
    P = P.reshape(C_DIM, B)
    # softmax is shift-invariant so beta_b cancels; no max-sub needed (|E|<=~8)
    C = (P / SE).T.reshape(B, 1, C_DIM)
    return C.astype(np.float32)


# revision 8
# speedup vs baseline: 2.7103x; 1.0658x over previous
"""Trainium2 Bass kernel for nn_AttentionMechanism (tanh-MLP attention).

Math (per batch b):
  q[:, b]   = W_h_w @ h_t[b] + W_h_b + W_b                  (host, tiny)
  U[beta,s,b] = sum_c W_w[beta,c] V[c,s,b]                   (PE)
  T = tanh(U + q)     (q folded in as the ACT per-partition bias)
  E[s,b]    = sum_beta bw[beta] T[beta,s,b]                  (PE, output replicated over partitions)
  w = exp(E)          (no max-subtraction needed: |E| <= ||bw||_1 ~ 8)
  P[c,b]    = sum_s w[s,b] V[c,s,b]                          (DVE mul+reduce, 2x mode)
  SE[b]     = sum_s w[s,b]                                   (DVE reduce)
  C[b,0,c]  = sum_cores P / sum_cores SE                     (host, tiny)

Sharding: position-parallel across 8 cores (hp dim, 8 rows each); softmax
combined on host.  Host pre-lays V out per-core as [c, b, s] bf16 (the
sharding-prep copy), so the device DMA reads contiguous runs at full HBM
bandwidth, every matmul rhs is s-contiguous (full PE rate), and the DVE
P stage runs in 2x mode with contiguous reduces.

On-chip: V lives in 16 resident tiles [c-chunk x b-octet] of [128, 8*512]
bf16; compute pipelines over b-groups of 4 while later octets DMA.  Each
matmul is N=512 = one batch x all 512 positions, PSUM tile [128, 4*512]
per beta-chunk covering a b-group; tanh reads each per-batch bank with
bias=q[beta-chunk, b] (fp32).  E uses a column-replicated beta_w lhsT so
exp directly yields partition-replicated w.
"""

import sys
from contextlib import ExitStack

import numpy as np

if "/opt/trn_rl_repo" not in sys.path:
    sys.path.insert(0, "/opt/trn_rl_repo")

import ml_dtypes

BF16 = ml_dtypes.bfloat16

HP, WP, C_DIM, B = 64, 64, 256, 64
BETA, HIDDEN = 512, 512
NCORES = 8
S_CORE = (HP // NCORES) * WP  # 512 positions per core
B_OCT = 8                     # batches per DMA tile
B_G = 4                       # batches per PSUM group / matmul group

_NC_CACHE = {}


def _build_nc(s_core=S_CORE):
    import concourse.bass as bass
    import concourse.bacc as bacc
    import concourse.tile as tile
    import concourse.mybir as mybir
    from concourse.mybir import dt

    AF = mybir.ActivationFunctionType
    ALU = mybir.AluOpType
    AX = mybir.AxisListType
    f32, bf16 = dt.float32, dt.bfloat16

    n_oct = B // B_OCT            # 8 DMA octets per c-chunk
    n_g = B // B_G                # 16 b-groups

    nc = bacc.Bacc("TRN2", target_bir_lowering=False, debug=False,
                   num_devices=NCORES)

    v_d = nc.dram_tensor("v", [C_DIM, B, s_core], bf16, kind="ExternalInput")
    wt_d = nc.dram_tensor("wt", [128, 2 * BETA], bf16, kind="ExternalInput")
    qs_d = nc.dram_tensor("qs", [128, 4 * B], f32, kind="ExternalInput")
    bwr_d = nc.dram_tensor("bwr", [128, BETA], bf16, kind="ExternalInput")
    p_d = nc.dram_tensor("p_out", [2, 128, B], f32, kind="ExternalOutput")
    se_d = nc.dram_tensor("se_out", [1, B], f32, kind="ExternalOutput")

    with tile.TileContext(nc) as tc, ExitStack() as ctx:
        cpool = ctx.enter_context(tc.tile_pool(name="const", bufs=1))
        vpool = ctx.enter_context(tc.tile_pool(name="vp", bufs=1))
        tpool = ctx.enter_context(tc.tile_pool(name="tp", bufs=5))
        wpool = ctx.enter_context(tc.tile_pool(name="wp", bufs=2))
        ppool = ctx.enter_context(tc.tile_pool(name="pp", bufs=2))
        apool = ctx.enter_context(tc.tile_pool(name="ap", bufs=1))
        psum = ctx.enter_context(tc.tile_pool(name="ps", bufs=2, space="PSUM"))

        # ---- constants ----
        wt_sb = cpool.tile([128, 2 * BETA], bf16, tag="wt")
        nc.sync.dma_start(wt_sb, wt_d[:])
        qs_sb = cpool.tile([128, 4 * B], f32, tag="qs")
        nc.sync.dma_start(qs_sb, qs_d[:])
        bwr_sb = cpool.tile([128, BETA], bf16, tag="bwr")
        nc.sync.dma_start(bwr_sb, bwr_d[:])

        # ---- V tiles: [c-chunk][b-octet] resident, DMA'd in octet order ----
        vv = [[None, None] for _ in range(n_oct)]
        for o in range(n_oct):
            for k in range(2):
                t = vpool.tile([128, B_OCT * s_core], bf16, tag=f"v{k}o{o}",
                               name=f"v{k}o{o}")
                nc.sync.dma_start(
                    t, v_d[k * 128:(k + 1) * 128, o * B_OCT:(o + 1) * B_OCT, :])
                vv[o][k] = t.rearrange("p (b s) -> p b s", s=s_core)

        # ---- output accumulators ----
        p_fin = [apool.tile([128, B], f32, tag=f"pfin{k}", name=f"pfin{k}")
                 for k in range(2)]
        se_fin = apool.tile([128, B], f32, tag="sefin")

        for g in range(n_g):
            b_base = g * B_G
            o = b_base // B_OCT            # octet index
            h = (b_base % B_OCT) // B_G    # half-within-octet
            t_tiles = []
            for m in range(4):
                u = psum.tile([128, 4 * 512], f32, tag="acc", name="u")
                for kp in range(2):
                    for b in range(B_G):
                        nc.tensor.matmul(
                            u[:, b * 512:(b + 1) * 512],
                            wt_sb[:, kp * BETA + m * 128:
                                  kp * BETA + (m + 1) * 128],
                            vv[o][kp][:, h * B_G + b, :],
                            start=(kp == 0), stop=(kp == 1))
                t_m = tpool.tile([128, 4 * 512], bf16, tag="t", name="t_m")
                for b in range(B_G):
                    nc.scalar.activation(
                        t_m[:, b * 512:(b + 1) * 512],
                        u[:, b * 512:(b + 1) * 512], AF.Tanh,
                        bias=qs_sb[:, m * B + b_base + b:m * B + b_base + b + 1])
                t_tiles.append(t_m)

            e_rep = psum.tile([128, 4 * 512], f32, tag="acc", name="e_rep")
            for m in range(4):
                for b in range(B_G):
                    nc.tensor.matmul(
                        e_rep[:, b * 512:(b + 1) * 512],
                        bwr_sb[:, m * 128:(m + 1) * 128],
                        t_tiles[m][:, b * 512:(b + 1) * 512],
                        start=(m == 0), stop=(m == 3))
            w_rep = wpool.tile([128, 4 * 512], bf16, tag="w", name="w_rep")
            nc.scalar.activation(w_rep, e_rep, AF.Exp)

            wv = w_rep.rearrange("p (b s) -> p b s", b=B_G)
            for k in range(2):
                prod = ppool.tile([128, B_G * s_core], bf16, tag="prod",
                                  name="prod")
                pv = prod.rearrange("p (b s) -> p b s", b=B_G)
                nc.vector.tensor_mul(
                    pv, vv[o][k][:, h * B_G:(h + 1) * B_G, :], wv)
                nc.vector.tensor_reduce(
                    p_fin[k][:, b_base:b_base + B_G], pv,
                    axis=AX.X, op=ALU.add)
            nc.vector.tensor_reduce(
                se_fin[:, b_base:b_base + B_G], wv, axis=AX.X, op=ALU.add)

        for k in range(2):
            nc.sync.dma_start(p_d[k], p_fin[k])
        nc.sync.dma_start(se_d[:], se_fin[0:1, :])

    nc.compile()
    return nc


def _get_nc(s_core=S_CORE):
    if s_core not in _NC_CACHE:
        _NC_CACHE[s_core] = _build_nc(s_core)
    return _NC_CACHE[s_core]


def _host_smalls(h_t, W_h_w, W_h_b, W_w, W_b, beta_w):
    q = h_t[:, 0, :].astype(np.float64) @ W_h_w.T.astype(np.float64) \
        + W_h_b + W_b                                  # [b, beta]
    # qs[p, m*64+b] = q[b, m*128+p]
    qs = np.ascontiguousarray(
        q.T.reshape(4, 128, B).transpose(1, 0, 2).reshape(128, 4 * B)
    ).astype(np.float32)
    wt = np.ascontiguousarray(
        W_w.T.reshape(2, 128, BETA).transpose(1, 0, 2).reshape(128, 2 * BETA)
    ).astype(BF16)
    bw = beta_w[0].astype(np.float32)
    bwr = np.ascontiguousarray(
        np.repeat(bw.reshape(4, 128).T[:, :, None], 128, axis=2).reshape(128, BETA)
    ).astype(BF16)
    return qs, wt, bwr


_PROFILE = False
_LAST_PERF = {}


def kernel(**inputs):
    from concourse.bass_utils import run_bass_kernel_spmd

    V = np.asarray(inputs["V"], dtype=np.float32)
    h_t = np.asarray(inputs["h_t"], dtype=np.float32)
    W_h_w = np.asarray(inputs["W_h_w"], dtype=np.float32)
    W_h_b = np.asarray(inputs["W_h_b"], dtype=np.float32)
    W_w = np.asarray(inputs["W_w"], dtype=np.float32)
    W_b = np.asarray(inputs["W_b"], dtype=np.float32)
    beta_w = np.asarray(inputs["beta_w"], dtype=np.float32)
    beta_b = np.asarray(inputs["beta_b"], dtype=np.float32)

    qs, wt, bwr = _host_smalls(h_t, W_h_w, W_h_b, W_w, W_b, beta_w)

    rows = HP // NCORES
    Vb = V.astype(BF16)
    in_maps = []
    for k in range(NCORES):
        # [s, c, b] -> [c, b, s] contiguous (per-core shard layout)
        vk = np.ascontiguousarray(
            Vb[k * rows:(k + 1) * rows].reshape(S_CORE, C_DIM, B)
            .transpose(1, 2, 0))
        in_maps.append({"v": vk, "wt": wt, "qs": qs, "bwr": bwr})

    nc = _get_nc()
    res = run_bass_kernel_spmd(nc, in_maps, core_ids=list(range(NCORES)),
                               trace=_PROFILE)
    if _PROFILE:
        _LAST_PERF["exec_time_ns"] = res.exec_time_ns
        _LAST_PERF["trace"] = res.instructions_and_trace
    P = np.zeros((2, 128, B), np.float64)
    SE = np.zeros((B,), np.float64)
    for r in res.results:
        P += r["p_out"]
        SE += r["se_out"][0]
    P = P.reshape(C_DIM, B)
    # softmax is shift-invariant so beta_b cancels; no max-sub needed (|E|<=~8)
    C = (P / SE).T.reshape(B, 1, C_DIM)
    return C.astype(np.float32)


# revision 9
# speedup vs baseline: 3.2027x; 1.1817x over previous
"""Trainium2 Bass kernel for nn_AttentionMechanism (tanh-MLP attention).

Math (per batch b):
  q[:, b]   = W_h_w @ h_t[b] + W_h_b + W_b                  (host, tiny)
  U[beta,s,b] = sum_c W_w[beta,c] V[c,s,b]                   (PE)
  T = tanh(U + q)     (q folded in as the ACT per-partition bias)
  E[s,b]    = sum_beta bw[beta] T[beta,s,b]                  (PE, output replicated over partitions)
  w = exp(E)          (no max-subtraction needed: |E| <= ||bw||_1 ~ 8)
  P[c,b]    = sum_s w[s,b] V[c,s,b]                          (DVE mul+reduce, 2x mode)
  SE[b]     = sum_s w[s,b]                                   (DVE reduce)
  C[b,0,c]  = sum_cores P / sum_cores SE                     (host, tiny)

Sharding: position-parallel across 8 cores (hp dim, 8 rows each); softmax
combined on host.  Host pre-lays V out per-core as [c, b, s] bf16 (the
sharding-prep copy), so the device DMA reads contiguous runs at full HBM
bandwidth, every matmul rhs is s-contiguous (full PE rate), and the DVE
P stage runs in 2x mode with contiguous reduces.

On-chip: V lives in 16 resident tiles [c-chunk x b-octet] of [128, 8*512]
bf16; compute pipelines over b-groups of 4 while later octets DMA.  Each
matmul is N=512 = one batch x all 512 positions, PSUM tile [128, 4*512]
per beta-chunk covering a b-group; tanh reads each per-batch bank with
bias=q[beta-chunk, b] (fp32).  E uses a column-replicated beta_w lhsT so
exp directly yields partition-replicated w.
"""

import sys
from contextlib import ExitStack

import numpy as np

if "/opt/trn_rl_repo" not in sys.path:
    sys.path.insert(0, "/opt/trn_rl_repo")

import ml_dtypes

BF16 = ml_dtypes.bfloat16

HP, WP, C_DIM, B = 64, 64, 256, 64
BETA, HIDDEN = 512, 512
NCORES = 8
S_CORE = (HP // NCORES) * WP  # 512 positions per core
B_OCT = 4                     # batches per DMA tile
B_G = 2                       # batches per PSUM group / matmul group

_NC_CACHE = {}


def _build_nc(s_core=S_CORE):
    import concourse.bass as bass
    import concourse.bacc as bacc
    import concourse.tile as tile
    import concourse.mybir as mybir
    from concourse.mybir import dt

    AF = mybir.ActivationFunctionType
    ALU = mybir.AluOpType
    AX = mybir.AxisListType
    f32, bf16 = dt.float32, dt.bfloat16

    n_oct = B // B_OCT            # 8 DMA octets per c-chunk
    n_g = B // B_G                # 16 b-groups

    nc = bacc.Bacc("TRN2", target_bir_lowering=False, debug=False,
                   num_devices=NCORES)

    v_d = nc.dram_tensor("v", [C_DIM, B, s_core], bf16, kind="ExternalInput")
    wt_d = nc.dram_tensor("wt", [128, 2 * BETA], bf16, kind="ExternalInput")
    qs_d = nc.dram_tensor("qs", [128, 4 * B], f32, kind="ExternalInput")
    bwr_d = nc.dram_tensor("bwr", [128, BETA], bf16, kind="ExternalInput")
    p_d = nc.dram_tensor("p_out", [2, 128, B], f32, kind="ExternalOutput")
    se_d = nc.dram_tensor("se_out", [1, B], f32, kind="ExternalOutput")

    with tile.TileContext(nc) as tc, ExitStack() as ctx:
        cpool = ctx.enter_context(tc.tile_pool(name="const", bufs=1))
        vpool = ctx.enter_context(tc.tile_pool(name="vp", bufs=1))
        tpool = ctx.enter_context(tc.tile_pool(name="tp", bufs=5))
        wpool = ctx.enter_context(tc.tile_pool(name="wp", bufs=2))
        ppool = ctx.enter_context(tc.tile_pool(name="pp", bufs=2))
        apool = ctx.enter_context(tc.tile_pool(name="ap", bufs=1))
        psum = ctx.enter_context(tc.tile_pool(name="ps", bufs=4, space="PSUM"))

        # ---- constants ----
        wt_sb = cpool.tile([128, 2 * BETA], bf16, tag="wt")
        nc.sync.dma_start(wt_sb, wt_d[:])
        qs_sb = cpool.tile([128, 4 * B], f32, tag="qs")
        nc.sync.dma_start(qs_sb, qs_d[:])
        bwr_sb = cpool.tile([128, BETA], bf16, tag="bwr")
        nc.sync.dma_start(bwr_sb, bwr_d[:])

        # ---- V tiles: [c-chunk][b-octet] resident, DMA'd in octet order ----
        vv = [[None, None] for _ in range(n_oct)]
        for o in range(n_oct):
            for k in range(2):
                t = vpool.tile([128, B_OCT * s_core], bf16, tag=f"v{k}o{o}",
                               name=f"v{k}o{o}")
                nc.sync.dma_start(
                    t, v_d[k * 128:(k + 1) * 128, o * B_OCT:(o + 1) * B_OCT, :])
                vv[o][k] = t.rearrange("p (b s) -> p b s", s=s_core)

        # ---- output accumulators ----
        p_fin = [apool.tile([128, B], f32, tag=f"pfin{k}", name=f"pfin{k}")
                 for k in range(2)]
        se_fin = apool.tile([128, B], f32, tag="sefin")

        for g in range(n_g):
            b_base = g * B_G
            o = b_base // B_OCT            # octet index
            h = (b_base % B_OCT) // B_G    # half-within-octet
            t_tiles = []
            for m in range(4):
                u = psum.tile([128, B_G * 512], f32, tag="acc", name="u")
                for kp in range(2):
                    for b in range(B_G):
                        nc.tensor.matmul(
                            u[:, b * 512:(b + 1) * 512],
                            wt_sb[:, kp * BETA + m * 128:
                                  kp * BETA + (m + 1) * 128],
                            vv[o][kp][:, h * B_G + b, :],
                            start=(kp == 0), stop=(kp == 1))
                t_m = tpool.tile([128, B_G * 512], bf16, tag="t", name="t_m")
                for b in range(B_G):
                    nc.scalar.activation(
                        t_m[:, b * 512:(b + 1) * 512],
                        u[:, b * 512:(b + 1) * 512], AF.Tanh,
                        bias=qs_sb[:, m * B + b_base + b:m * B + b_base + b + 1])
                t_tiles.append(t_m)

            e_rep = psum.tile([128, B_G * 512], f32, tag="acc", name="e_rep")
            for m in range(4):
                for b in range(B_G):
                    nc.tensor.matmul(
                        e_rep[:, b * 512:(b + 1) * 512],
                        bwr_sb[:, m * 128:(m + 1) * 128],
                        t_tiles[m][:, b * 512:(b + 1) * 512],
                        start=(m == 0), stop=(m == 3))
            w_rep = wpool.tile([128, B_G * 512], bf16, tag="w", name="w_rep")
            nc.scalar.activation(w_rep, e_rep, AF.Exp)

            wv = w_rep.rearrange("p (b s) -> p b s", b=B_G)
            for k in range(2):
                prod = ppool.tile([128, B_G * s_core], bf16, tag="prod",
                                  name="prod")
                pv = prod.rearrange("p (b s) -> p b s", b=B_G)
                nc.vector.tensor_mul(
                    pv, vv[o][k][:, h * B_G:(h + 1) * B_G, :], wv)
                nc.vector.tensor_reduce(
                    p_fin[k][:, b_base:b_base + B_G], pv,
                    axis=AX.X, op=ALU.add)
            nc.vector.tensor_reduce(
                se_fin[:, b_base:b_base + B_G], wv, axis=AX.X, op=ALU.add)

        for k in range(2):
            nc.sync.dma_start(p_d[k], p_fin[k])
        nc.sync.dma_start(se_d[:], se_fin[0:1, :])

    nc.compile()
    return nc


def _get_nc(s_core=S_CORE):
    if s_core not in _NC_CACHE:
        _NC_CACHE[s_core] = _build_nc(s_core)
    return _NC_CACHE[s_core]


def _host_smalls(h_t, W_h_w, W_h_b, W_w, W_b, beta_w):
    q = h_t[:, 0, :].astype(np.float64) @ W_h_w.T.astype(np.float64) \
        + W_h_b + W_b                                  # [b, beta]
    # qs[p, m*64+b] = q[b, m*128+p]
    qs = np.ascontiguousarray(
        q.T.reshape(4, 128, B).transpose(1, 0, 2).reshape(128, 4 * B)
    ).astype(np.float32)
    wt = np.ascontiguousarray(
        W_w.T.reshape(2, 128, BETA).transpose(1, 0, 2).reshape(128, 2 * BETA)
    ).astype(BF16)
    bw = beta_w[0].astype(np.float32)
    bwr = np.ascontiguousarray(
        np.repeat(bw.reshape(4, 128).T[:, :, None], 128, axis=2).reshape(128, BETA)
    ).astype(BF16)
    return qs, wt, bwr


_PROFILE = False
_LAST_PERF = {}


def kernel(**inputs):
    from concourse.bass_utils import run_bass_kernel_spmd

    V = np.asarray(inputs["V"], dtype=np.float32)
    h_t = np.asarray(inputs["h_t"], dtype=np.float32)
    W_h_w = np.asarray(inputs["W_h_w"], dtype=np.float32)
    W_h_b = np.asarray(inputs["W_h_b"], dtype=np.float32)
    W_w = np.asarray(inputs["W_w"], dtype=np.float32)
    W_b = np.asarray(inputs["W_b"], dtype=np.float32)
    beta_w = np.asarray(inputs["beta_w"], dtype=np.float32)
    beta_b = np.asarray(inputs["beta_b"], dtype=np.float32)

    qs, wt, bwr = _host_smalls(h_t, W_h_w, W_h_b, W_w, W_b, beta_w)

    rows = HP // NCORES
    Vb = V.astype(BF16)
    in_maps = []
    for k in range(NCORES):
        # [s, c, b] -> [c, b, s] contiguous (per-core shard layout)
        vk = np.ascontiguousarray(
            Vb[k * rows:(k + 1) * rows].reshape(S_CORE, C_DIM, B)
            .transpose(1, 2, 0))
        in_maps.append({"v": vk, "wt": wt, "qs": qs, "bwr": bwr})

    nc = _get_nc()
    res = run_bass_kernel_spmd(nc, in_maps, core_ids=list(range(NCORES)),
                               trace=_PROFILE)
    if _PROFILE:
        _LAST_PERF["exec_time_ns"] = res.exec_time_ns
        _LAST_PERF["trace"] = res.instructions_and_trace
    P = np.zeros((2, 128, B), np.float64)
    SE = np.zeros((B,), np.float64)
    for r in res.results:
        P += r["p_out"]
        SE += r["se_out"][0]
    P = P.reshape(C_DIM, B)
    # softmax is shift-invariant so beta_b cancels; no max-sub needed (|E|<=~8)
    C = (P / SE).T.reshape(B, 1, C_DIM)
    return C.astype(np.float32)
